# revision 2
# baseline (speedup 1.0000x reference)
"""Trainium2 Bass kernel for nn_LlamaAttention_48816598286577.

Llama attention with block-streaming sparse mask (sink=1 block, local
window=8 blocks, BLOCK=128), B=1 S=2048 H=4096, 32 q heads / 8 kv heads,
head_dim 128, non-interleaved RoPE.

Sharding: tensor-parallel over heads across 8 cores (4 q heads + 1 kv
head per core). All compute in bf16 (PSUM accumulates f32).

v2 structure (vs the phase-separated baseline): the sequence is processed
in 8 windows of 256 positions (= one query-block pair each). Window w
computes the QKV projections + RoPE for its 256 positions in two passes
(pass A: q0,q1,k; pass B: q2,q3,v) so projections only hold 3 PSUM banks.
Attention for pair w-1 is interleaved, a few matmuls at a time, into
window w's projection stream, and its AllGather is issued at the end of
window w -- ~150us earlier than the old end-of-phase-1 schedule, so the
CC stream (which costs ~20us per AllGather and ~88us for the first op
after idle) runs concurrently with the projections instead of serializing
the o_proj endgame.  o_proj runs as a tail, consuming the gathered
attention outputs through XBAR transpose DMAs (so no PE transposes of the
attention outputs are needed; V transposes stay on the PE).

PSUM budget (8 banks): 3 proj + 2 score-groups (shared with V transpose)
+ 1 PV accumulator + 2 o_proj accumulators.
"""

import functools
from collections import deque

import numpy as np

import concourse.bass as bass
import concourse.mybir as mybir
import concourse.tile as tile
from concourse import bacc
from concourse.bass_utils import run_bass_kernel_spmd

# problem constants (hardcoded per contract)
B, S, H = 1, 2048, 4096
NQ, NKV, HD = 32, 8, 128
BLOCK = 128
NBLK = S // BLOCK          # 16
SINK_BLOCKS = 1
LOCAL_BLOCKS = 8
ROPE_BASE = 10000.0
N_CORES = 8
HQ = NQ // N_CORES         # 4 q heads per core
DQ = HQ * HD               # 512 q columns per core
SCALE = 1.0 / float(np.sqrt(HD))

KC = H // 128              # 32 contraction chunks for projections
NPAIR = NBLK // 2          # 8 query pairs of 256
WW = 256                   # window width = one pair of q blocks

F32 = mybir.dt.float32
BF16 = mybir.dt.bfloat16

VB = 129                   # v-block stride in vNat (128 v cols + ones col)


def _pair_blocks(i: int):
    """Key blocks for query pair i with per-block subblock coverage."""
    out = []
    for j in range(2 * i + 2):
        left = j <= 2 * i and (2 * i - j < LOCAL_BLOCKS or j < SINK_BLOCKS)
        right = j <= 2 * i + 1 and (2 * i + 1 - j < LOCAL_BLOCKS or j < SINK_BLOCKS)
        if left or right:
            out.append((j, left, right))
    return out


def _groups(blocks, widths):
    """Greedy score groups of total width <= 512 (one PSUM bank)."""
    out = []
    g = 0
    while g < len(blocks):
        g_end, gw = g, 0
        while g_end < len(blocks) and gw + widths[g_end] <= 512:
            gw += widths[g_end]
            g_end += 1
        out.append((g, g_end, gw))
        g = g_end
    return out


class _IL:
    """Round-robin generator interleaver: pump() emits one quantum."""

    def __init__(self):
        self.q = deque()

    def add(self, gen):
        self.q.append(gen)

    def pump(self):
        while self.q:
            try:
                next(self.q[0])
                return True
            except StopIteration:
                self.q.popleft()
        return False

    def drain(self):
        while self.pump():
            pass


def build_nc():
    nc = bacc.Bacc(
        "TRN2", target_bir_lowering=False, debug=False, num_devices=N_CORES
    )
    hid_sw = nc.dram_tensor("hid_sw", [128, KC * S], BF16, kind="ExternalInput").ap()
    wq_sw = nc.dram_tensor("wq_sw", [128, KC * DQ], BF16, kind="ExternalInput").ap()
    wk_sw = nc.dram_tensor("wk_sw", [128, KC * HD], BF16, kind="ExternalInput").ap()
    wv_sw = nc.dram_tensor("wv_sw", [128, KC * HD], BF16, kind="ExternalInput").ap()
    wo_sw = nc.dram_tensor("wo_sw", [128, KC * DQ], BF16, kind="ExternalInput").ap()
    cosF = nc.dram_tensor("cosF", [128, S], F32, kind="ExternalInput").ap()
    sinS = nc.dram_tensor("sinS", [128, S], F32, kind="ExternalInput").ap()
    tri = nc.dram_tensor("tri", [128, 128], BF16, kind="ExternalInput").ap()
    eye = nc.dram_tensor("eye", [128, 128], BF16, kind="ExternalInput").ap()
    out = nc.dram_tensor("out", [S, DQ], F32, kind="ExternalOutput").ap()

    hid_r = hid_sw.rearrange("p (c s) -> p c s", c=KC)

    with tile.TileContext(nc) as tc:
        with (
            tc.tile_pool(name="persist", bufs=1) as pp,
            tc.tile_pool(name="dram", bufs=1, space="DRAM") as dramp,
        ):
            # ---- persistent SBUF state
            qTr = [
                [
                    pp.tile([128, 2 * WW], BF16, tag=f"qTr{h}_{nq}", name=f"qTr{h}_{nq}")
                    for nq in range(4)
                ]
                for h in range(HQ)
            ]
            kTr = [
                pp.tile([128, 2 * WW], BF16, tag=f"kTr{nq}", name=f"kTr{nq}")
                for nq in range(4)
            ]
            vNat = [
                pp.tile([128, 4 * VB], BF16, tag=f"vNat{nq}", name=f"vNat{nq}")
                for nq in range(4)
            ]
            tri_sb = pp.tile([128, 128], BF16, tag="tri", name="tri_sb")
            eye_sb = pp.tile([128, 128], BF16, tag="eye", name="eye_sb")
            wq_sb = pp.tile([128, KC * DQ], BF16, tag="wq", name="wq_sb")
            wk_sb = pp.tile([128, KC * HD], BF16, tag="wk", name="wk_sb")
            wv_sb = pp.tile([128, KC * HD], BF16, tag="wv", name="wv_sb")
            wo_sb = pp.tile([128, KC * DQ], BF16, tag="wo", name="wo_sb")
            cos_all = pp.tile([128, S], F32, tag="cos", name="cos_all")
            sin_all = pp.tile([128, S], F32, tag="sin", name="sin_all")

            # ---- DRAM collective buffers: natural [q, hd] layout
            ag_ins = [
                dramp.tile([2 * 128, DQ], BF16, tag=f"agin{c}", name=f"agin{c}")
                for c in range(NPAIR)
            ]
            ag_outs = [
                dramp.tile(
                    [N_CORES * 2 * 128, DQ], BF16, tag=f"agout{c}", name=f"agout{c}",
                    addr_space="Shared",
                )
                for c in range(NPAIR)
            ]

            # Warm up the CC stream immediately (first collective pays a
            # ~88us barrier; keep the stream busy until the first real AG).
            warm_in = dramp.tile([128, 8], BF16, tag="win", name="warm_in")
            warm_sb = pp.tile([128, 8], BF16, tag="wsb", name="warm_sb")
            nc.vector.memset(warm_sb[:], 0.0)
            nc.sync.dma_start(warm_in[:], warm_sb[:])
            warm_outs = [
                dramp.tile(
                    [N_CORES * 128, 8], BF16, tag=f"wout{w}", name=f"warm_out{w}",
                    addr_space="Shared",
                )
                for w in range(2)
            ]
            for w in range(2):
                nc.gpsimd.collective_compute(
                    "AllGather",
                    mybir.AluOpType.bypass,
                    replica_groups=[list(range(N_CORES))],
                    ins=[warm_in.opt()],
                    outs=[warm_outs[w].opt()],
                )

            nc.sync.dma_start(eye_sb[:], eye[:])
            nc.sync.dma_start(tri_sb[:], tri[:])
            for nq in range(4):
                for b in range(4):
                    nc.vector.memset(vNat[nq][:, b * VB + 128 : b * VB + 129], 1.0)

            with (
                tc.tile_pool(name="hidp", bufs=2) as hidp,
                tc.tile_pool(name="small", bufs=2) as sp,
                tc.tile_pool(name="ep", bufs=3) as ep,
                tc.tile_pool(name="asb", bufs=4) as asb,
                tc.tile_pool(name="agp", bufs=4) as agp,
                tc.tile_pool(name="evp", bufs=2) as evp,
                tc.tile_pool(name="pjp", bufs=1, space="PSUM") as pjp,
                tc.tile_pool(name="sgp", bufs=2, space="PSUM") as sgp,
                tc.tile_pool(name="onp", bufs=1, space="PSUM") as onp,
                tc.tile_pool(name="opp", bufs=1, space="PSUM") as opp,
            ):
                # ---------- staging: weights + window-0 hid interleaved
                hid_tiles = {}
                h0 = hidp.tile([128, KC * WW], BF16, tag="hid", name="hid_w0")
                hid_tiles[0] = h0
                h0_r = h0.rearrange("p (c s) -> p c s", c=KC)
                bounds = [0, 1, 2, 4, 6, 8] + list(range(12, KC + 1, 4))
                pieces = list(zip(bounds[:-1], bounds[1:]))
                for (a, b) in pieces:
                    nc.sync.dma_start(
                        wq_sb[:, a * DQ : b * DQ], wq_sw[:, a * DQ : b * DQ]
                    )
                    nc.sync.dma_start(
                        wk_sb[:, a * HD : b * HD], wk_sw[:, a * HD : b * HD]
                    )
                    nc.sync.dma_start(
                        wv_sb[:, a * HD : b * HD], wv_sw[:, a * HD : b * HD]
                    )
                    nc.sync.dma_start(h0_r[:, a:b, :], hid_r[:, a:b, 0:WW])
                nc.sync.dma_start(cos_all[:], cosF[:])
                nc.sync.dma_start(sin_all[:], sinS[:])

                il = _IL()
                ag_issued = [False] * NPAIR

                def issue_ag(p):
                    nc.gpsimd.collective_compute(
                        "AllGather",
                        mybir.AluOpType.bypass,
                        replica_groups=[list(range(N_CORES))],
                        ins=[ag_ins[p].opt()],
                        outs=[ag_outs[p].opt()],
                    )
                    ag_issued[p] = True

                def attn_unit(p, h):
                    """Generator: attention for pair p, head h, in quanta."""
                    q0 = p * WW
                    qq = q0 // 512
                    qbase = q0 - qq * 512
                    blocks = _pair_blocks(p)
                    widths = [256 if (l and r) else 128 for (_, l, r) in blocks]
                    offs = list(np.cumsum([0] + widths))
                    e_t = ep.tile([128, 2304], BF16, tag="e", name="e_t")

                    for (g, g_end, gw) in _groups(blocks, widths):
                        s_grp = sgp.tile([128, 512], F32, tag="sg", name="s_grp")
                        for bi in range(g, g_end):
                            j, l, r = blocks[bi]
                            qs = qbase if l else qbase + 128
                            w_ = widths[bi]
                            o = offs[bi] - offs[g]
                            nc.tensor.matmul(
                                s_grp[:, o : o + w_],
                                kTr[j // 4][:, (j % 4) * 128 : (j % 4 + 1) * 128],
                                qTr[h][qq][:, qs : qs + w_],
                                start=True,
                                stop=True,
                            )
                        nc.scalar.activation(
                            e_t[:, offs[g] : offs[g] + gw],
                            s_grp[:, 0:gw],
                            mybir.ActivationFunctionType.Exp,
                            scale=SCALE,
                        )
                        for bi in range(g, g_end):
                            j, l, r = blocks[bi]
                            if j == 2 * p:
                                nc.vector.tensor_mul(
                                    e_t[:, offs[bi] : offs[bi] + 128],
                                    e_t[:, offs[bi] : offs[bi] + 128],
                                    tri_sb[:],
                                )
                            elif j == 2 * p + 1:
                                o2 = offs[bi] + widths[bi] - 128
                                nc.vector.tensor_mul(
                                    e_t[:, o2 : o2 + 128],
                                    e_t[:, o2 : o2 + 128],
                                    tri_sb[:],
                                )
                        yield

                    o_nat = onp.tile([128, 2 * VB], F32, tag="on", name="o_nat")
                    nL = sum(1 for (_, l, _) in blocks if l)
                    cL = 0
                    for bi, (j, l, r) in enumerate(blocks):
                        if not l:
                            continue
                        mv = vNat[j // 4][:, (j % 4) * VB : (j % 4) * VB + VB]
                        nc.tensor.matmul(
                            o_nat[:, 0:VB],
                            e_t[:, offs[bi] : offs[bi] + 128],
                            mv,
                            start=(cL == 0),
                            stop=(cL == nL - 1),
                        )
                        cL += 1
                    yield

                    nR = sum(1 for (_, _, r) in blocks if r)
                    cR = 0
                    for bi, (j, l, r) in enumerate(blocks):
                        if not r:
                            continue
                        mv = vNat[j // 4][:, (j % 4) * VB : (j % 4) * VB + VB]
                        o = offs[bi] + (widths[bi] - 128)
                        nc.tensor.matmul(
                            o_nat[:, VB : 2 * VB],
                            e_t[:, o : o + 128],
                            mv,
                            start=(cR == 0),
                            stop=(cR == nR - 1),
                        )
                        cR += 1
                    # finalize: normalize rows, ship natural-layout [q, d]
                    r_sb = asb.tile([128, 2], F32, tag="r", name="r_sb", bufs=4)
                    nc.vector.reciprocal(r_sb[:, 0:1], o_nat[:, 128:129])
                    nc.vector.reciprocal(
                        r_sb[:, 1:2], o_nat[:, 2 * VB - 1 : 2 * VB]
                    )
                    at_nat = asb.tile(
                        [128, 256], BF16, tag="an", name="at_nat", bufs=8
                    )
                    nc.vector.tensor_scalar_mul(
                        at_nat[:, 0:128], o_nat[:, 0:128], r_sb[:, 0:1]
                    )
                    nc.vector.tensor_scalar_mul(
                        at_nat[:, 128:256], o_nat[:, VB : VB + 128], r_sb[:, 1:2]
                    )
                    nc.sync.dma_start(
                        ag_ins[p][0:128, h * 128 : (h + 1) * 128], at_nat[:, 0:128]
                    )
                    nc.sync.dma_start(
                        ag_ins[p][128:256, h * 128 : (h + 1) * 128],
                        at_nat[:, 128:256],
                    )
                    yield

                def unit_quanta(p):
                    blocks = _pair_blocks(p)
                    widths = [256 if (l and r) else 128 for (_, l, r) in blocks]
                    return len(_groups(blocks, widths)) + 3

                # ---------- RoPE per window pass
                def rope_pass(w, srcs):
                    """srcs: list of (idx, psum_tile, dstT_slice)."""
                    cw = slice(w * WW, (w + 1) * WW)
                    raws = []
                    for idx, ps_x, dstT in srcs:
                        raw = sp.tile(
                            [128, WW], BF16, tag=f"raw{idx}", name=f"raw{idx}"
                        )
                        nc.vector.tensor_copy(raw[:], ps_x[:])  # sole PSUM read
                        raws.append(raw)
                    for (idx, ps_x, dstT), raw in zip(srcs, raws):
                        swp = sp.tile(
                            [128, WW], BF16, tag=f"swp{idx}", name=f"swp{idx}"
                        )
                        nc.sync.dma_start(swp[0:64, :], raw[64:128, :])
                        nc.sync.dma_start(swp[64:128, :], raw[0:64, :])
                        t1 = sp.tile([128, WW], BF16, tag=f"t1_{idx}", name=f"t1_{idx}")
                        nc.vector.tensor_mul(t1[:], raw[:], cos_all[:, cw])
                        t2 = sp.tile([128, WW], BF16, tag="t2", name="t2", bufs=4)
                        nc.vector.tensor_mul(t2[:], swp[:], sin_all[:, cw])
                        nc.vector.tensor_add(dstT[:], t1[:], t2[:])

                # ---------- window loop
                deferred_pe = []  # V transposes deferred into next window

                for w in range(NPAIR):
                    if w + 1 < NPAIR:
                        h2 = hidp.tile(
                            [128, KC * WW], BF16, tag="hid", name=f"hid_w{w+1}"
                        )
                        h2_r = h2.rearrange("p (c s) -> p c s", c=KC)
                        nc.sync.dma_start(
                            h2_r[:], hid_r[:, :, (w + 1) * WW : (w + 2) * WW]
                        )
                        hid_tiles[w + 1] = h2
                    if w < 4:
                        # trickle wo in quarters during windows 0-3
                        q = KC * DQ // 4
                        nc.sync.dma_start(
                            wo_sb[:, w * q : (w + 1) * q], wo_sw[:, w * q : (w + 1) * q]
                        )

                    hid_c = hid_tiles.pop(w)
                    npend = 4 * unit_quanta(w - 1) if w >= 1 else 0
                    # pump positions among the 64 chunk-passes (start late
                    # enough that RoPE of window w-1 has landed)
                    positions = {}
                    if npend:
                        span = 64 - 6
                        for k in range(npend):
                            pos = 5 + (k * span) // npend
                            positions[pos] = positions.get(pos, 0) + 1

                    qq, half = w // 2, w % 2
                    cp = 0
                    for pas in range(2):
                        ha, hb = (0, 1) if pas == 0 else (2, 3)
                        ps_a = pjp.tile([128, WW], F32, tag="pa0", name="ps_a")
                        ps_b = pjp.tile([128, WW], F32, tag="pa1", name="ps_b")
                        ps_kv = pjp.tile([128, WW], F32, tag="pk", name="ps_kv")
                        wkv_sb = wk_sb if pas == 0 else wv_sb
                        for c in range(KC):
                            st, sp_ = (c == 0), (c == KC - 1)
                            hs = hid_c[:, c * WW : (c + 1) * WW]
                            nc.tensor.matmul(
                                ps_a[:],
                                wq_sb[:, c * DQ + ha * HD : c * DQ + (ha + 1) * HD],
                                hs, start=st, stop=sp_,
                            )
                            nc.tensor.matmul(
                                ps_b[:],
                                wq_sb[:, c * DQ + hb * HD : c * DQ + (hb + 1) * HD],
                                hs, start=st, stop=sp_,
                            )
                            nc.tensor.matmul(
                                ps_kv[:], wkv_sb[:, c * HD : (c + 1) * HD], hs,
                                start=st, stop=sp_,
                            )
                            if cp == 2 and deferred_pe:
                                for fn in deferred_pe:
                                    fn()
                                deferred_pe = []
                            for _ in range(positions.get(cp, 0)):
                                il.pump()
                            cp += 1

                        dsl = slice(half * WW, (half + 1) * WW)
                        if pas == 0:
                            srcs = [
                                (2, ps_kv, kTr[qq][:, dsl]),
                                (0, ps_a, qTr[0][qq][:, dsl]),
                                (1, ps_b, qTr[1][qq][:, dsl]),
                            ]
                            rope_pass(w, srcs)
                        else:
                            srcs = [
                                (3, ps_a, qTr[2][qq][:, dsl]),
                                (4, ps_b, qTr[3][qq][:, dsl]),
                            ]
                            rope_pass(w, srcs)
                            # V: evacuate + 2 PE transposes (deferred into
                            # the next window so the PE never waits here)
                            vT_q = sp.tile([128, WW], BF16, tag="vT", name="vT_q")
                            nc.vector.tensor_copy(vT_q[:], ps_kv[:])

                            def v_tr(w=w, vT_q=vT_q, qq=qq, half=half):
                                for b_ in range(2):
                                    tr = sgp.tile(
                                        [128, 128], BF16, tag="sg", name="tr"
                                    )
                                    nc.tensor.transpose(
                                        tr[:],
                                        vT_q[:, b_ * 128 : (b_ + 1) * 128],
                                        eye_sb[:],
                                    )
                                    blk = 2 * half + b_
                                    nc.vector.tensor_copy(
                                        vNat[qq][:, blk * VB : blk * VB + 128], tr[:]
                                    )

                            deferred_pe.append(v_tr)

                    if w >= 1:
                        il.drain()
                        issue_ag(w - 1)
                    for h in range(HQ):
                        if w + 1 < NPAIR:
                            il.add(attn_unit(w, h))

                # ---------- tail: pair-7 attention + o_proj
                for fn in deferred_pe:
                    fn()
                deferred_pe = []
                for h in range(HQ):
                    il.add(attn_unit(NPAIR - 1, h))

                ag_sbs = {}

                def oproj_dma(p, r):
                    tiles = []
                    for c2 in range(4):
                        t = agp.tile([128, 256], BF16, tag="ag", name="ag_sb", bufs=8)
                        nc.scalar.dma_start_transpose(
                            t[:],
                            ag_outs[p][256 * r : 256 * (r + 1), 128 * c2 : 128 * (c2 + 1)],
                        )
                        tiles.append(t)
                    ag_sbs[(p, r)] = tiles

                def oproj_mm(p, r, ps01):
                    tiles = ag_sbs.pop((p, r))
                    for c2 in range(4):
                        c = 4 * r + c2
                        for sb in range(2):
                            nc.tensor.matmul(
                                ps01[sb][:],
                                tiles[c2][:, sb * 128 : (sb + 1) * 128],
                                wo_sb[:, c * DQ : (c + 1) * DQ],
                                start=(c == 0),
                                stop=(c == KC - 1),
                            )

                def oproj_finish(p, ps01):
                    q0 = p * 256
                    for sb in range(2):
                        ev = evp.tile([128, DQ], F32, tag="ev", name="ev")
                        nc.vector.tensor_copy(ev[:], ps01[sb][:])
                        nc.sync.dma_start(
                            out[q0 + sb * 128 : q0 + (sb + 1) * 128, :], ev[:]
                        )

                seq = [(p, r) for p in range(NPAIR) for r in range(N_CORES)]
                oproj_dma(*seq[0])
                oproj_dma(*seq[1])
                ps_map = {}
                for k, (p, r) in enumerate(seq):
                    if k + 2 < len(seq):
                        oproj_dma(*seq[k + 2])
                    if r == 0:
                        ps_map[p] = [
                            opp.tile([128, DQ], F32, tag=f"op{sb}", name=f"op{sb}")
                            for sb in range(2)
                        ]
                    oproj_mm(p, r, ps_map[p])
                    il.pump()
                    il.pump()
                    if not il.q and not ag_issued[NPAIR - 1]:
                        issue_ag(NPAIR - 1)
                    if r == N_CORES - 1:
                        oproj_finish(p, ps_map.pop(p))
                il.drain()
                assert ag_issued[NPAIR - 1]

    nc.compile()
    return nc


@functools.lru_cache(maxsize=1)
def _cached_nc():
    return build_nc()


def _tables():
    pos = np.arange(S, dtype=np.float64)
    inv = 1.0 / (ROPE_BASE ** (np.arange(0, HD, 2, dtype=np.float64) / HD))  # [64]
    f = inv[:, None] * pos[None, :]                   # [64, S]
    cos = np.cos(f).astype(np.float32)
    sin = np.sin(f).astype(np.float32)
    cosF = np.concatenate([cos, cos], axis=0)         # [128, S]
    sinS = np.concatenate([-sin, sin], axis=0)        # [128, S]
    k_idx = np.arange(128)[:, None]
    q_idx = np.arange(128)[None, :]
    tri = (k_idx <= q_idx).astype(np.float32)         # [k, q] causal in-block
    return cosF, sinS, tri


def _swz(w: np.ndarray, bf16) -> np.ndarray:
    """[KC*128, W] -> chunk-major [128, KC*W] bf16."""
    kc, w_ = w.shape[0] // 128, w.shape[1]
    return np.ascontiguousarray(
        w.reshape(kc, 128, w_).transpose(1, 0, 2).reshape(128, kc * w_)
    ).astype(bf16)


def _run(hidden_states, wq, wk, wv, wo, **run_kwargs):
    nc = _cached_nc()
    bf16 = mybir.dt.np(BF16)
    # hid_sw[p, c*S + s] = hidden[s, c*128 + p]
    hid2 = np.asarray(hidden_states, dtype=np.float32).reshape(S, H)
    hid_sw = np.ascontiguousarray(
        hid2.reshape(S, KC, 128).transpose(2, 1, 0).reshape(128, KC * S)
    ).astype(bf16)
    cosF, sinS, tri = _tables()
    in_maps = []
    for c in range(N_CORES):
        in_maps.append(
            {
                "hid_sw": hid_sw,
                "wq_sw": _swz(wq[:, c * DQ : (c + 1) * DQ], bf16),
                "wk_sw": _swz(wk[:, c * HD : (c + 1) * HD], bf16),
                "wv_sw": _swz(wv[:, c * HD : (c + 1) * HD], bf16),
                "wo_sw": _swz(wo[:, c * DQ : (c + 1) * DQ], bf16),
                "cosF": cosF,
                "sinS": sinS,
                "tri": tri.astype(bf16),
                "eye": np.eye(128, dtype=np.float32).astype(bf16),
            }
        )
    res = run_bass_kernel_spmd(
        nc, in_maps, core_ids=list(range(N_CORES)), **run_kwargs
    )
    full = np.concatenate(
        [res.results[r]["out"] for r in range(N_CORES)], axis=1
    )
    return full.reshape(B, S, H).astype(np.float32), res


def kernel(hidden_states, wq, wk, wv, wo):
    out, _ = _run(hidden_states, wq, wk, wv, wo)
    return out


# revision 6
# speedup vs baseline: 1.6642x; 1.6642x over previous
"""Trainium2 Bass kernel for nn_LlamaAttention_48816598286577.

Llama attention with block-streaming sparse mask (sink=1 block, local
window=8 blocks, BLOCK=128), B=1 S=2048 H=4096, 32 q heads / 8 kv heads,
head_dim 128, non-interleaved RoPE.

Sharding: tensor-parallel over heads across 8 cores (4 q heads + 1 kv
head per core). All compute in bf16 (PSUM accumulates f32).

v2 structure (vs the phase-separated baseline): the sequence is processed
in 8 windows of 256 positions (= one query-block pair each). Window w
computes the QKV projections + RoPE for its 256 positions in two passes
(pass A: q0,q1,k; pass B: q2,q3,v) so projections only hold 3 PSUM banks.
Attention for pair w-1 is interleaved, a few matmuls at a time, into
window w's projection stream, and its AllGather is issued at the end of
window w -- ~150us earlier than the old end-of-phase-1 schedule, so the
CC stream (which costs ~20us per AllGather and ~88us for the first op
after idle) runs concurrently with the projections instead of serializing
the o_proj endgame.  o_proj runs as a tail, consuming the gathered
attention outputs through XBAR transpose DMAs (so no PE transposes of the
attention outputs are needed; V transposes stay on the PE).

PSUM budget (8 banks): 3 proj + 2 score-groups (shared with V transpose)
+ 1 PV accumulator + 2 o_proj accumulators.
"""

import functools
from collections import deque

import numpy as np

import concourse.bass as bass
import concourse.mybir as mybir
import concourse.tile as tile
from concourse import bacc
from concourse.bass_utils import run_bass_kernel_spmd

# problem constants (hardcoded per contract)
B, S, H = 1, 2048, 4096
NQ, NKV, HD = 32, 8, 128
BLOCK = 128
NBLK = S // BLOCK          # 16
SINK_BLOCKS = 1
LOCAL_BLOCKS = 8
ROPE_BASE = 10000.0
N_CORES = 8
HQ = NQ // N_CORES         # 4 q heads per core
DQ = HQ * HD               # 512 q columns per core
SCALE = 1.0 / float(np.sqrt(HD))

KC = H // 128              # 32 contraction chunks for projections
NPAIR = NBLK // 2          # 8 query pairs of 256
WW = 256                   # window width = one pair of q blocks

F32 = mybir.dt.float32
BF16 = mybir.dt.bfloat16

VB = 129                   # v-block stride in vNat (128 v cols + ones col)


def _pair_blocks(i: int):
    """Key blocks for query pair i with per-block subblock coverage."""
    out = []
    for j in range(2 * i + 2):
        left = j <= 2 * i and (2 * i - j < LOCAL_BLOCKS or j < SINK_BLOCKS)
        right = j <= 2 * i + 1 and (2 * i + 1 - j < LOCAL_BLOCKS or j < SINK_BLOCKS)
        if left or right:
            out.append((j, left, right))
    return out


def _groups(blocks, widths):
    """Greedy score groups of total width <= 512 (one PSUM bank)."""
    out = []
    g = 0
    while g < len(blocks):
        g_end, gw = g, 0
        while g_end < len(blocks) and gw + widths[g_end] <= 512:
            gw += widths[g_end]
            g_end += 1
        out.append((g, g_end, gw))
        g = g_end
    return out


class _IL:
    """Round-robin generator interleaver: pump() emits one quantum."""

    def __init__(self):
        self.q = deque()

    def add(self, gen):
        self.q.append(gen)

    def pump(self):
        while self.q:
            try:
                next(self.q[0])
                return True
            except StopIteration:
                self.q.popleft()
        return False

    def drain(self):
        while self.pump():
            pass


def build_nc():
    nc = bacc.Bacc(
        "TRN2", target_bir_lowering=False, debug=False, num_devices=N_CORES
    )
    hid_sw = nc.dram_tensor("hid_sw", [128, KC * S], BF16, kind="ExternalInput").ap()
    wq_sw = nc.dram_tensor("wq_sw", [128, KC * DQ], BF16, kind="ExternalInput").ap()
    wk_sw = nc.dram_tensor("wk_sw", [128, KC * HD], BF16, kind="ExternalInput").ap()
    wv_sw = nc.dram_tensor("wv_sw", [128, KC * HD], BF16, kind="ExternalInput").ap()
    wo_sw = nc.dram_tensor("wo_sw", [128, KC * DQ], BF16, kind="ExternalInput").ap()
    cosF = nc.dram_tensor("cosF", [128, S], F32, kind="ExternalInput").ap()
    sinS = nc.dram_tensor("sinS", [128, S], F32, kind="ExternalInput").ap()
    tri = nc.dram_tensor("tri", [128, 128], BF16, kind="ExternalInput").ap()
    eye = nc.dram_tensor("eye", [128, 128], BF16, kind="ExternalInput").ap()
    out = nc.dram_tensor("out", [S, DQ], F32, kind="ExternalOutput").ap()

    hid_r = hid_sw.rearrange("p (c s) -> p c s", c=KC)

    with tile.TileContext(nc) as tc:
        with (
            tc.tile_pool(name="persist", bufs=1) as pp,
            tc.tile_pool(name="dram", bufs=1, space="DRAM") as dramp,
        ):
            # ---- persistent SBUF state
            qTr = [
                [
                    pp.tile([128, 2 * WW], BF16, tag=f"qTr{h}_{nq}", name=f"qTr{h}_{nq}")
                    for nq in range(4)
                ]
                for h in range(HQ)
            ]
            kTr = [
                pp.tile([128, 2 * WW], BF16, tag=f"kTr{nq}", name=f"kTr{nq}")
                for nq in range(4)
            ]
            vNat = [
                pp.tile([128, 4 * VB], BF16, tag=f"vNat{nq}", name=f"vNat{nq}")
                for nq in range(4)
            ]
            tri_sb = pp.tile([128, 128], BF16, tag="tri", name="tri_sb")
            eye_sb = pp.tile([128, 128], BF16, tag="eye", name="eye_sb")
            wq_sb = pp.tile([128, KC * DQ], BF16, tag="wq", name="wq_sb")
            wk_sb = pp.tile([128, KC * HD], BF16, tag="wk", name="wk_sb")
            wv_sb = pp.tile([128, KC * HD], BF16, tag="wv", name="wv_sb")
            wo_sb = pp.tile([128, KC * DQ], BF16, tag="wo", name="wo_sb")
            cos_all = pp.tile([128, S], F32, tag="cos", name="cos_all")
            sin_all = pp.tile([128, S], F32, tag="sin", name="sin_all")

            # ---- DRAM collective buffers: [hd, q] layout (at transposed
            # on the PE before the AllGather)
            ag_ins = [
                dramp.tile([DQ, 256], BF16, tag=f"agin{c}", name=f"agin{c}")
                for c in range(NPAIR)
            ]
            ag_outs = [
                dramp.tile(
                    [H, 256], BF16, tag=f"agout{c}", name=f"agout{c}",
                    addr_space="Shared",
                )
                for c in range(NPAIR)
            ]

            # Warm up the CC stream immediately (first collective pays a
            # ~88us barrier; keep the stream busy until the first real AG).
            warm_in = dramp.tile([128, 8], BF16, tag="win", name="warm_in")
            warm_sb = pp.tile([128, 8], BF16, tag="wsb", name="warm_sb")
            nc.vector.memset(warm_sb[:], 0.0)
            nc.sync.dma_start(warm_in[:], warm_sb[:])
            warm_outs = [
                dramp.tile(
                    [N_CORES * 128, 8], BF16, tag=f"wout{w}", name=f"warm_out{w}",
                    addr_space="Shared",
                )
                for w in range(2)
            ]
            for w in range(2):
                nc.gpsimd.collective_compute(
                    "AllGather",
                    mybir.AluOpType.bypass,
                    replica_groups=[list(range(N_CORES))],
                    ins=[warm_in.opt()],
                    outs=[warm_outs[w].opt()],
                )

            nc.sync.dma_start(eye_sb[:], eye[:])
            nc.sync.dma_start(tri_sb[:], tri[:])
            for nq in range(4):
                for b in range(4):
                    nc.vector.memset(vNat[nq][:, b * VB + 128 : b * VB + 129], 1.0)

            with (
                tc.tile_pool(name="hidp", bufs=2) as hidp,
                tc.tile_pool(name="small", bufs=2) as sp,
                tc.tile_pool(name="ep", bufs=3) as ep,
                tc.tile_pool(name="asb", bufs=4) as asb,
                tc.tile_pool(name="agp", bufs=4) as agp,
                tc.tile_pool(name="evp", bufs=2) as evp,
                tc.tile_pool(name="pjp", bufs=1, space="PSUM") as pjp,
                tc.tile_pool(name="sgp", bufs=2, space="PSUM") as sgp,
                tc.tile_pool(name="onp", bufs=1, space="PSUM") as onp,
                tc.tile_pool(name="opp", bufs=1, space="PSUM") as opp,
            ):
                # ---------- staging: weights + window-0 hid interleaved
                hid_tiles = {}
                h0 = hidp.tile([128, KC * WW], BF16, tag="hid", name="hid_w0")
                hid_tiles[0] = h0
                h0_r = h0.rearrange("p (c s) -> p c s", c=KC)
                bounds = [0, 1, 2, 4, 6, 8] + list(range(12, KC + 1, 4))
                pieces = list(zip(bounds[:-1], bounds[1:]))
                for (a, b) in pieces:
                    nc.sync.dma_start(
                        wq_sb[:, a * DQ : b * DQ], wq_sw[:, a * DQ : b * DQ]
                    )
                    nc.sync.dma_start(
                        wk_sb[:, a * HD : b * HD], wk_sw[:, a * HD : b * HD]
                    )
                    nc.sync.dma_start(
                        wv_sb[:, a * HD : b * HD], wv_sw[:, a * HD : b * HD]
                    )
                    nc.sync.dma_start(h0_r[:, a:b, :], hid_r[:, a:b, 0:WW])
                nc.sync.dma_start(cos_all[:], cosF[:])
                nc.sync.dma_start(sin_all[:], sinS[:])

                il = _IL()
                ag_issued = [False] * NPAIR

                def issue_ag(p):
                    nc.gpsimd.collective_compute(
                        "AllGather",
                        mybir.AluOpType.bypass,
                        replica_groups=[list(range(N_CORES))],
                        ins=[ag_ins[p].opt()],
                        outs=[ag_outs[p].opt()],
                    )
                    ag_issued[p] = True

                def attn_unit(p, h):
                    """Generator: attention for pair p, head h, in quanta."""
                    q0 = p * WW
                    qq = q0 // 512
                    qbase = q0 - qq * 512
                    blocks = _pair_blocks(p)
                    widths = [256 if (l and r) else 128 for (_, l, r) in blocks]
                    offs = list(np.cumsum([0] + widths))
                    e_t = ep.tile([128, 2304], BF16, tag="e", name="e_t")

                    for (g, g_end, gw) in _groups(blocks, widths):
                        s_grp = sgp.tile([128, 512], F32, tag="sg", name="s_grp")
                        for bi in range(g, g_end):
                            j, l, r = blocks[bi]
                            qs = qbase if l else qbase + 128
                            w_ = widths[bi]
                            o = offs[bi] - offs[g]
                            nc.tensor.matmul(
                                s_grp[:, o : o + w_],
                                kTr[j // 4][:, (j % 4) * 128 : (j % 4 + 1) * 128],
                                qTr[h][qq][:, qs : qs + w_],
                                start=True,
                                stop=True,
                            )
                        nc.scalar.activation(
                            e_t[:, offs[g] : offs[g] + gw],
                            s_grp[:, 0:gw],
                            mybir.ActivationFunctionType.Exp,
                            scale=SCALE,
                        )
                        for bi in range(g, g_end):
                            j, l, r = blocks[bi]
                            if j == 2 * p:
                                nc.vector.tensor_mul(
                                    e_t[:, offs[bi] : offs[bi] + 128],
                                    e_t[:, offs[bi] : offs[bi] + 128],
                                    tri_sb[:],
                                )
                            elif j == 2 * p + 1:
                                o2 = offs[bi] + widths[bi] - 128
                                nc.vector.tensor_mul(
                                    e_t[:, o2 : o2 + 128],
                                    e_t[:, o2 : o2 + 128],
                                    tri_sb[:],
                                )
                        yield

                    o_nat = onp.tile([128, 2 * VB], F32, tag="on", name="o_nat")
                    nL = sum(1 for (_, l, _) in blocks if l)
                    cL = 0
                    for bi, (j, l, r) in enumerate(blocks):
                        if not l:
                            continue
                        mv = vNat[j // 4][:, (j % 4) * VB : (j % 4) * VB + VB]
                        nc.tensor.matmul(
                            o_nat[:, 0:VB],
                            e_t[:, offs[bi] : offs[bi] + 128],
                            mv,
                            start=(cL == 0),
                            stop=(cL == nL - 1),
                        )
                        cL += 1
                    yield

                    nR = sum(1 for (_, _, r) in blocks if r)
                    cR = 0
                    for bi, (j, l, r) in enumerate(blocks):
                        if not r:
                            continue
                        mv = vNat[j // 4][:, (j % 4) * VB : (j % 4) * VB + VB]
                        o = offs[bi] + (widths[bi] - 128)
                        nc.tensor.matmul(
                            o_nat[:, VB : 2 * VB],
                            e_t[:, o : o + 128],
                            mv,
                            start=(cR == 0),
                            stop=(cR == nR - 1),
                        )
                        cR += 1
                    # normalize rows in natural layout
                    r_sb = asb.tile([128, 2], F32, tag="r", name="r_sb", bufs=4)
                    nc.vector.reciprocal(r_sb[:, 0:1], o_nat[:, 128:129])
                    nc.vector.reciprocal(
                        r_sb[:, 1:2], o_nat[:, 2 * VB - 1 : 2 * VB]
                    )
                    at_nat = asb.tile(
                        [128, 256], BF16, tag="an", name="at_nat", bufs=8
                    )
                    nc.vector.tensor_scalar_mul(
                        at_nat[:, 0:128], o_nat[:, 0:128], r_sb[:, 0:1]
                    )
                    nc.vector.tensor_scalar_mul(
                        at_nat[:, 128:256], o_nat[:, VB : VB + 128], r_sb[:, 1:2]
                    )
                    yield

                    # transpose to [d, q] on the PE, ship to the AG buffer
                    trT = sgp.tile([128, 256], BF16, tag="sg", name="trT")
                    nc.tensor.transpose(trT[:, 0:128], at_nat[:, 0:128], eye_sb[:])
                    nc.tensor.transpose(
                        trT[:, 128:256], at_nat[:, 128:256], eye_sb[:]
                    )
                    at_cT = asb.tile(
                        [128, 256], BF16, tag="at", name="at_cT", bufs=8
                    )
                    nc.vector.tensor_copy(at_cT[:], trT[:])
                    nc.sync.dma_start(
                        ag_ins[p][h * 128 : (h + 1) * 128, :], at_cT[:]
                    )
                    yield

                def unit_quanta(p):
                    blocks = _pair_blocks(p)
                    widths = [256 if (l and r) else 128 for (_, l, r) in blocks]
                    return len(_groups(blocks, widths)) + 4

                # ---------- RoPE per window pass
                def rope_pass(w, srcs):
                    """srcs: list of (idx, psum_tile, dstT_slice)."""
                    cw = slice(w * WW, (w + 1) * WW)
                    raws = []
                    for idx, ps_x, dstT in srcs:
                        raw = sp.tile(
                            [128, WW], BF16, tag=f"raw{idx}", name=f"raw{idx}"
                        )
                        nc.vector.tensor_copy(raw[:], ps_x[:])  # sole PSUM read
                        raws.append(raw)
                    for (idx, ps_x, dstT), raw in zip(srcs, raws):
                        swp = sp.tile(
                            [128, WW], BF16, tag=f"swp{idx}", name=f"swp{idx}"
                        )
                        nc.sync.dma_start(swp[0:64, :], raw[64:128, :])
                        nc.sync.dma_start(swp[64:128, :], raw[0:64, :])
                        t1 = sp.tile([128, WW], BF16, tag=f"t1_{idx}", name=f"t1_{idx}")
                        nc.vector.tensor_mul(t1[:], raw[:], cos_all[:, cw])
                        t2 = sp.tile([128, WW], BF16, tag="t2", name="t2", bufs=4)
                        nc.vector.tensor_mul(t2[:], swp[:], sin_all[:, cw])
                        nc.vector.tensor_add(dstT[:], t1[:], t2[:])

                # ---------- window loop
                deferred_pe = []  # V transposes deferred into next window

                for w in range(NPAIR):
                    if w + 1 < NPAIR:
                        h2 = hidp.tile(
                            [128, KC * WW], BF16, tag="hid", name=f"hid_w{w+1}"
                        )
                        h2_r = h2.rearrange("p (c s) -> p c s", c=KC)
                        nc.sync.dma_start(
                            h2_r[:], hid_r[:, :, (w + 1) * WW : (w + 2) * WW]
                        )
                        hid_tiles[w + 1] = h2
                    if w < 4:
                        # trickle wo in quarters during windows 0-3
                        q = KC * DQ // 4
                        nc.sync.dma_start(
                            wo_sb[:, w * q : (w + 1) * q], wo_sw[:, w * q : (w + 1) * q]
                        )

                    hid_c = hid_tiles.pop(w)
                    npend = 4 * unit_quanta(w - 1) if w >= 1 else 0
                    # pump positions among the 64 chunk-passes (start late
                    # enough that RoPE of window w-1 has landed)
                    positions = {}
                    if npend:
                        span = 64 - 6
                        for k in range(npend):
                            pos = 5 + (k * span) // npend
                            positions[pos] = positions.get(pos, 0) + 1

                    qq, half = w // 2, w % 2
                    cp = 0
                    for pas in range(2):
                        ha, hb = (0, 1) if pas == 0 else (2, 3)
                        ps_a = pjp.tile([128, WW], F32, tag="pa0", name="ps_a")
                        ps_b = pjp.tile([128, WW], F32, tag="pa1", name="ps_b")
                        ps_kv = pjp.tile([128, WW], F32, tag="pk", name="ps_kv")
                        wkv_sb = wk_sb if pas == 0 else wv_sb
                        for c in range(KC):
                            st, sp_ = (c == 0), (c == KC - 1)
                            hs = hid_c[:, c * WW : (c + 1) * WW]
                            nc.tensor.matmul(
                                ps_a[:],
                                wq_sb[:, c * DQ + ha * HD : c * DQ + (ha + 1) * HD],
                                hs, start=st, stop=sp_,
                            )
                            nc.tensor.matmul(
                                ps_b[:],
                                wq_sb[:, c * DQ + hb * HD : c * DQ + (hb + 1) * HD],
                                hs, start=st, stop=sp_,
                            )
                            nc.tensor.matmul(
                                ps_kv[:], wkv_sb[:, c * HD : (c + 1) * HD], hs,
                                start=st, stop=sp_,
                            )
                            if cp == 2 and deferred_pe:
                                for fn in deferred_pe:
                                    fn()
                                deferred_pe = []
                            for _ in range(positions.get(cp, 0)):
                                il.pump()
                            cp += 1

                        dsl = slice(half * WW, (half + 1) * WW)
                        if pas == 0:
                            srcs = [
                                (2, ps_kv, kTr[qq][:, dsl]),
                                (0, ps_a, qTr[0][qq][:, dsl]),
                                (1, ps_b, qTr[1][qq][:, dsl]),
                            ]
                            rope_pass(w, srcs)
                        else:
                            srcs = [
                                (3, ps_a, qTr[2][qq][:, dsl]),
                                (4, ps_b, qTr[3][qq][:, dsl]),
                            ]
                            rope_pass(w, srcs)
                            # V: evacuate + 2 PE transposes (deferred into
                            # the next window so the PE never waits here)
                            vT_q = sp.tile([128, WW], BF16, tag="vT", name="vT_q")
                            nc.vector.tensor_copy(vT_q[:], ps_kv[:])

                            def v_tr(w=w, vT_q=vT_q, qq=qq, half=half):
                                for b_ in range(2):
                                    tr = sgp.tile(
                                        [128, 128], BF16, tag="sg", name="tr"
                                    )
                                    nc.tensor.transpose(
                                        tr[:],
                                        vT_q[:, b_ * 128 : (b_ + 1) * 128],
                                        eye_sb[:],
                                    )
                                    blk = 2 * half + b_
                                    nc.vector.tensor_copy(
                                        vNat[qq][:, blk * VB : blk * VB + 128], tr[:]
                                    )

                            deferred_pe.append(v_tr)

                    if w >= 1:
                        il.drain()
                        issue_ag(w - 1)
                    for h in range(HQ):
                        if w + 1 < NPAIR:
                            il.add(attn_unit(w, h))

                # ---------- tail: pair-7 attention + o_proj
                for fn in deferred_pe:
                    fn()
                deferred_pe = []
                for h in range(HQ):
                    il.add(attn_unit(NPAIR - 1, h))

                ag_sbs = {}

                def oproj_dma(p, r):
                    # plain loads on the GpSimd (SWDGE) queue: nothing
                    # latency-critical shares that queue, so its waits on
                    # AllGather completion can't block exp/RoPE/hid DMAs.
                    tiles = []
                    for c2 in range(4):
                        c = 4 * r + c2
                        t = agp.tile([128, 256], BF16, tag="ag", name="ag_sb", bufs=8)
                        nc.gpsimd.dma_start(
                            t[:], ag_outs[p][128 * c : 128 * (c + 1), :]
                        )
                        tiles.append(t)
                    ag_sbs[(p, r)] = tiles

                def oproj_mm(p, r, ps01):
                    tiles = ag_sbs.pop((p, r))
                    for c2 in range(4):
                        c = 4 * r + c2
                        for sb in range(2):
                            nc.tensor.matmul(
                                ps01[sb][:],
                                tiles[c2][:, sb * 128 : (sb + 1) * 128],
                                wo_sb[:, c * DQ : (c + 1) * DQ],
                                start=(c == 0),
                                stop=(c == KC - 1),
                            )

                def oproj_finish(p, ps01):
                    q0 = p * 256
                    for sb in range(2):
                        ev = evp.tile([128, DQ], F32, tag="ev", name="ev")
                        nc.vector.tensor_copy(ev[:], ps01[sb][:])
                        nc.sync.dma_start(
                            out[q0 + sb * 128 : q0 + (sb + 1) * 128, :], ev[:]
                        )

                seq = [(p, r) for p in range(NPAIR) for r in range(N_CORES)]
                oproj_dma(*seq[0])
                oproj_dma(*seq[1])
                ps_map = {}
                for k, (p, r) in enumerate(seq):
                    if k + 2 < len(seq):
                        oproj_dma(*seq[k + 2])
                    if r == 0:
                        ps_map[p] = [
                            opp.tile([128, DQ], F32, tag=f"op{sb}", name=f"op{sb}")
                            for sb in range(2)
                        ]
                    oproj_mm(p, r, ps_map[p])
                    il.pump()
                    il.pump()
                    if not il.q and not ag_issued[NPAIR - 1]:
                        issue_ag(NPAIR - 1)
                    if r == N_CORES - 1:
                        oproj_finish(p, ps_map.pop(p))
                il.drain()
                assert ag_issued[NPAIR - 1]

    nc.compile()
    return nc


@functools.lru_cache(maxsize=1)
def _cached_nc():
    return build_nc()


def _tables():
    pos = np.arange(S, dtype=np.float64)
    inv = 1.0 / (ROPE_BASE ** (np.arange(0, HD, 2, dtype=np.float64) / HD))  # [64]
    f = inv[:, None] * pos[None, :]                   # [64, S]
    cos = np.cos(f).astype(np.float32)
    sin = np.sin(f).astype(np.float32)
    cosF = np.concatenate([cos, cos], axis=0)         # [128, S]
    sinS = np.concatenate([-sin, sin], axis=0)        # [128, S]
    k_idx = np.arange(128)[:, None]
    q_idx = np.arange(128)[None, :]
    tri = (k_idx <= q_idx).astype(np.float32)         # [k, q] causal in-block
    return cosF, sinS, tri


def _swz(w: np.ndarray, bf16) -> np.ndarray:
    """[KC*128, W] -> chunk-major [128, KC*W] bf16."""
    kc, w_ = w.shape[0] // 128, w.shape[1]
    return np.ascontiguousarray(
        w.reshape(kc, 128, w_).transpose(1, 0, 2).reshape(128, kc * w_)
    ).astype(bf16)


def _run(hidden_states, wq, wk, wv, wo, **run_kwargs):
    nc = _cached_nc()
    bf16 = mybir.dt.np(BF16)
    # hid_sw[p, c*S + s] = hidden[s, c*128 + p]
    hid2 = np.asarray(hidden_states, dtype=np.float32).reshape(S, H)
    hid_sw = np.ascontiguousarray(
        hid2.reshape(S, KC, 128).transpose(2, 1, 0).reshape(128, KC * S)
    ).astype(bf16)
    cosF, sinS, tri = _tables()
    in_maps = []
    for c in range(N_CORES):
        in_maps.append(
            {
                "hid_sw": hid_sw,
                "wq_sw": _swz(wq[:, c * DQ : (c + 1) * DQ], bf16),
                "wk_sw": _swz(wk[:, c * HD : (c + 1) * HD], bf16),
                "wv_sw": _swz(wv[:, c * HD : (c + 1) * HD], bf16),
                "wo_sw": _swz(wo[:, c * DQ : (c + 1) * DQ], bf16),
                "cosF": cosF,
                "sinS": sinS,
                "tri": tri.astype(bf16),
                "eye": np.eye(128, dtype=np.float32).astype(bf16),
            }
        )
    res = run_bass_kernel_spmd(
        nc, in_maps, core_ids=list(range(N_CORES)), **run_kwargs
    )
    full = np.concatenate(
        [res.results[r]["out"] for r in range(N_CORES)], axis=1
    )
    return full.reshape(B, S, H).astype(np.float32), res


def kernel(hidden_states, wq, wk, wv, wo):
    out, _ = _run(hidden_states, wq, wk, wv, wo)
    return out
